# revision 41
# baseline (speedup 1.0000x reference)
"""Trainium2 Bass kernel for causal multi-head attention (B=2, S=2048, E=1024, H=16).

Sharding: 8 cores = 2 batches x 4 head-groups (4 heads each).
Each core computes its batch's QKV for its 4 heads, causal attention, and a
partial output projection; host sums the 4 group partials per batch + b_out.

All big matmuls run in float32r (TF32-like, 1 cycle/row at N>=256).
"""
import sys

sys.path.insert(0, "/opt/trn_rl_repo")

from contextlib import ExitStack

import numpy as np

import concourse.bass as bass
import concourse.tile as tile
from concourse import bacc, mybir
from concourse.bass_utils import run_bass_kernel_spmd

dt = mybir.dt

B, S, E, H = 2, 2048, 1024, 16
HD = 64                     # head dim
HPC = 4                     # heads per core
NC = 8                      # cores
KE = E // 128               # 8 contraction k-tiles for projections
NT = S // 128               # 16 token tiles
NCH = S // 512              # 4 token chunks
FQK = 512                   # q+k features per core (4 heads * 64 * 2)
FV = 256                    # v features per core

# engine used for fp32 -> fp32r rounding copies of DMA'd inputs
ROUND_ENGINE = "gpsimd"


def _build_program():
    nc = bacc.Bacc("TRN2", target_bir_lowering=False, debug=False, num_devices=NC)

    xT_d = nc.dram_tensor("xT", [E, S], dt.float32, kind="ExternalInput")
    wqkT_d = nc.dram_tensor("wqkT", [E, FQK], dt.float32, kind="ExternalInput")
    wvT_d = nc.dram_tensor("wvT", [E, FV], dt.float32, kind="ExternalInput")
    bqk_d = nc.dram_tensor("bqk", [FQK], dt.float32, kind="ExternalInput")
    bv_d = nc.dram_tensor("bv", [FV], dt.float32, kind="ExternalInput")
    wo_d = nc.dram_tensor("wo", [FV, E], dt.float32, kind="ExternalInput")
    mask_d = nc.dram_tensor("trimask", [128, 128], dt.float32, kind="ExternalInput")
    y_d = nc.dram_tensor("y", [S, E], dt.float32, kind="ExternalOutput")

    with TileKernel(nc) as tk:
        tk.build(xT_d, wqkT_d, wvT_d, bqk_d, bv_d, wo_d, mask_d, y_d)
    nc.compile()
    return nc


class TileKernel:
    def __init__(self, nc):
        self.nc = nc
        self.ctx = ExitStack()
        self.tc_cm = tile.TileContext(nc)

    def __enter__(self):
        self.tc = self.tc_cm.__enter__()
        return self

    def __exit__(self, *a):
        self.ctx.close()
        return self.tc_cm.__exit__(*a)

    _round_i = 0

    def round_copy(self, out, in_):
        nc = self.nc
        nc.vector.tensor_copy(out, in_)

    def build(self, xT_d, wqkT_d, wvT_d, bqk_d, bv_d, wo_d, mask_d, y_d):
        nc, tc, ctx = self.nc, self.tc, self.ctx
        pool = lambda name, bufs, **kw: ctx.enter_context(
            tc.tile_pool(name=name, bufs=bufs, **kw)
        )

        const_p = pool("const", 1)
        xs_p = pool("xs", 1)
        xr_p = pool("xr", 2)
        qkt_p = pool("qkt", 1)
        vones_p = pool("vones", 1)
        attn_p = pool("attn", 3)
        pair_p = pool("pair", 1)
        small_p = pool("small", 1)
        y_p = pool("y", 4)
        # PSUM: ps (2 banks x 2 bufs) + po (1 bank x 1 buf x 4 tags) = 8
        ps_p = pool("ps", 2, space="PSUM")
        po_p = pool("po", 1, space="PSUM")
        p1_p = ps_p  # qkv/outproj psums share the ps slots

        # ---- weights ----

        wstage_cm = tc.tile_pool(name="wstage", bufs=1)
        wstage_p = wstage_cm.__enter__()
        wqk_st = wstage_p.tile([128, KE * FQK], dt.float32, tag="wst")
        wqk_big = const_p.tile([128, KE * FQK], dt.float32r, tag="wqk")
        for h in range(2):
            hs = slice(h * (KE // 2) * FQK, (h + 1) * (KE // 2) * FQK)
            nc.sync.dma_start(
                wqk_st[:, hs].rearrange("p (ke f) -> p ke f", f=FQK),
                wqkT_d[h * 512 : (h + 1) * 512, :].rearrange("(ke p) f -> p ke f", p=128),
            )
            self.round_copy(wqk_big[:, hs], wqk_st[:, hs])
        wqk_r = [wqk_big[:, FQK * ke : FQK * (ke + 1)] for ke in range(KE)]

        wv_st = wstage_p.tile([128, KE * FV], dt.float32, tag="wst")
        nc.sync.dma_start(
            wv_st[:].rearrange("p (ke f) -> p ke f", f=FV),
            wvT_d[:].rearrange("(ke p) f -> p ke f", p=128),
        )
        wv_big = const_p.tile([128, KE * FV], dt.float32r, tag="wv")
        self.round_copy(wv_big[:], wv_st[:])
        wv_r = [wv_big[:, FV * ke : FV * (ke + 1)] for ke in range(KE)]

        wo_st = wstage_p.tile([128, 2 * E], dt.float32, tag="wst")
        nc.sync.dma_start(
            wo_st[:].rearrange("p (kt f) -> p kt f", f=E),
            wo_d[:].rearrange("(kt p) f -> p kt f", p=128),
        )
        wo_big = const_p.tile([128, 2 * E], dt.float32r, tag="wo")
        self.round_copy(wo_big[:], wo_st[:])
        wo_r = [wo_big[:, E * kt : E * (kt + 1)] for kt in range(2)]
        wstage_cm.__exit__(None, None, None)

        bqk_sb = const_p.tile([128, 4], dt.float32, tag="bqk")
        nc.sync.dma_start(bqk_sb[:], bqk_d[:].rearrange("(f p) -> p f", p=128))
        bv_sb = const_p.tile([128, 2], dt.float32, tag="bv")
        nc.sync.dma_start(bv_sb[:], bv_d[:].rearrange("(f p) -> p f", p=128))
        ones_sb = const_p.tile([128, 1, 1], dt.float32, tag="ones")
        nc.vector.memset(ones_sb[:], 1.0)
        mask_sb = const_p.tile([128, 128], dt.float32, tag="mask")
        nc.sync.dma_start(mask_sb[:], mask_d[:])

        # ---- persistent activations ----
        # qkt tiles: 0: q heads 0,1 | 1: q heads 2,3 | 2: k heads 0,1 | 3: k heads 2,3
        qkt = [qkt_p.tile([128, S], dt.float32r, tag=f"qkt{f}", name=f"qkt{f}") for f in range(4)]
        # vones[t]: [v h0 |1| v h1 |1| v h2 |1| v h3 |1] for token tile t
        vones = [vones_p.tile([128, 4 * 65], dt.float32r, tag=f"v{t}", name=f"v{t}") for t in range(NT)]
        # pair tiles: final normalized attn output, [head dims x 2, S]
        pairt = [pair_p.tile([128, S], dt.float32r, tag=f"pair{hp}", name=f"pair{hp}") for hp in range(2)]

        env = dict(
            xT_d=xT_d, wqk_r=wqk_r, wv_r=wv_r, bqk_sb=bqk_sb, ones_sb=ones_sb,
            xs_p=xs_p, xr_p=xr_p, p1_p=p1_p, qkt=qkt, vones=vones,
            ps_p=ps_p, po_p=po_p, attn_p=attn_p, small_p=small_p,
            pairt=pairt, bv_sb=bv_sb, mask_sb=mask_sb, wo_r=wo_r,
            y_p=y_p, y_d=y_d,
        )
        # startup: chunk-0 qkv emitted directly
        for u in self.qkv_units(0, env):
            u()
        for c in range(NCH):
            fillers = []
            if c + 1 < NCH:
                fillers += self.qkv_units(c + 1, env)
            tail = self.oproj_units(c - 1, env) if c >= 1 else []
            self.attention_chunk(c, env, fillers, tail)
        for u in self.oproj_units(NCH - 1, env):
            u()

    # ------------------------------------------------------------------
    def qkv_units(self, c, env):
        nc = self.nc
        cs = slice(512 * c, 512 * (c + 1))
        xT_d, wqk_r, wv_r = env["xT_d"], env["wqk_r"], env["wv_r"]
        qkt, vones = env["qkt"], env["vones"]
        bqk_sb, ones_sb = env["bqk_sb"], env["ones_sb"]
        xs_p, xr_p, p1_p = env["xs_p"], env["xr_p"], env["p1_p"]

        xs = xs_p.tile([128, KE * 512], dt.float32, tag="xs", name="xs")
        nc.sync.dma_start(
            xs[:].rearrange("p (ke f) -> p ke f", f=512),
            xT_d[:, cs].rearrange("(ke p) f -> p ke f", p=128),
        )
        xrb = xr_p.tile([128, KE * 512], dt.float32r, tag="xr", name="xrb")
        xr = [xrb[:, 512 * ke : 512 * (ke + 1)] for ke in range(KE)]
        units = []
        for ke in range(KE):
            units.append(lambda ke=ke: self.round_copy(
                xrb[:, 512 * ke : 512 * (ke + 1)], xs[:, 512 * ke : 512 * (ke + 1)]))

        def qk_unit(f):
            pq = p1_p.tile([128, 1024], dt.float32, tag="ps", name="pq")
            for ke in range(KE):
                nc.tensor.matmul(
                    pq[:, 0:512], wqk_r[ke][:, 128 * f : 128 * (f + 1)], xr[ke][:],
                    start=(ke == 0), stop=(ke == KE - 1),
                )
            nc.vector.tensor_scalar_add(qkt[f][:, cs], pq[:, 0:512], bqk_sb[:, f : f + 1])

        def v_unit(t4):
            t = 4 * c + t4
            pv = p1_p.tile([128, 1024], dt.float32, tag="ps", name="pv")
            for ke in range(KE):
                nc.tensor.matmul(
                    pv[:, 0:FV],
                    xr[ke][:, 128 * t4 : 128 * (t4 + 1)], wv_r[ke][:],
                    start=(ke == 0), stop=(ke == KE - 1),
                )
            vt = vones[t]
            v3 = vt[:].rearrange("p (g d) -> p g d", d=65)
            nc.vector.tensor_copy(
                v3[:, :, 0:64],
                pv[:, 0:FV].rearrange("p (g d) -> p g d", d=64),
            )
            nc.vector.tensor_copy(v3[:, :, 64:65], ones_sb[:].to_broadcast((128, 4, 1)))

        for f in range(4):
            units.append(lambda f=f: qk_unit(f))
        for t4 in range(4):
            units.append(lambda t4=t4: v_unit(t4))
        return units

    # ------------------------------------------------------------------
    def oproj_units(self, c, env):
        nc = self.nc
        pairt, wo_r, p1_p, y_p, y_d = (
            env["pairt"], env["wo_r"], env["p1_p"], env["y_p"], env["y_d"])
        units = []
        ysbs = {}

        def unit(t4, o):
            t = 4 * c + t4
            if o == 0:
                ysbs[t4] = y_p.tile([128, E], dt.float32, tag="y", name="ysb")
            ysb = ysbs[t4]
            py = p1_p.tile([128, 1024], dt.float32, tag="ps", name="py")
            for kt in range(2):
                nc.tensor.matmul(
                    py[:, 0:512],
                    pairt[kt][:, 128 * t : 128 * (t + 1)],
                    wo_r[kt][:, 512 * o : 512 * (o + 1)],
                    start=(kt == 0), stop=(kt == 1),
                )
            if o == 0:
                nc.vector.tensor_copy(ysb[:, 0:512], py[:, 0:512])
            else:
                nc.scalar.activation(
                    ysb[:, 512:1024], py[:, 0:512], mybir.ActivationFunctionType.Copy
                )
                eng = nc.gpsimd if t % 2 == 0 else nc.sync
                eng.dma_start(y_d[128 * t : 128 * (t + 1), :], ysb[:])

        for t4 in range(4):
            for o in range(2):
                units.append(lambda t4=t4, o=o: unit(t4, o))
        return units

    # ------------------------------------------------------------------
    def attention_chunk(self, c, env, fillers, tail=()):
        """Attention for both head pairs of chunk c, weaving filler units
        (next-chunk qkv / prev-chunk out-proj) into the PE stream."""
        nc = self.nc
        qkt, vones = env["qkt"], env["vones"]
        ps_p, po_p, attn_p, small_p = (
            env["ps_p"], env["po_p"], env["attn_p"], env["small_p"])
        pairt, bv_sb, mask_sb = env["pairt"], env["bv_sb"], env["mask_sb"]
        nj = 4 * c + 4
        # po[2*hp + h_idx]: [65, 512] accumulator per head
        po = [po_p.tile([65, 512], dt.float32, tag=f"po{i}", name=f"po{i}")
              for i in range(4)]

        nfill = len(fillers)
        iters = 2 * nj
        emitted = 0

        def emit_pv(hp, j, off, at):
            for h_idx in range(2):
                slot = 2 * hp + h_idx
                nc.tensor.matmul(
                    po[slot][:, off:512],
                    vones[j][:, 65 * slot : 65 * slot + 65],
                    at[:, 512 * h_idx + off : 512 * (h_idx + 1)],
                    start=(j == 0), stop=(j == nj - 1),
                    skip_group_check=True,
                )

        it = 0
        for hp in range(2):
            pending = []
            for j in range(nj):
                ps = ps_p.tile([128, 1024], dt.float32, tag="ps", name="ps")
                at = attn_p.tile([128, 1024], dt.float32r, tag="attn", name="at")
                m = j - 4 * c
                off = 128 * m if 1 <= m <= 3 else 0
                off_mm = off if m in (1, 2) else 0
                for h_idx in range(2):
                    r0 = 64 * h_idx
                    nc.tensor.matmul(
                        ps[:, 512 * h_idx + off_mm : 512 * (h_idx + 1)],
                        qkt[2 + hp][r0 : r0 + 64, 128 * j : 128 * (j + 1)],
                        qkt[hp][r0 : r0 + 64, 512 * c + off_mm : 512 * (c + 1)],
                        start=True, stop=True,
                    )
                if m >= 0:
                    for h_idx in range(2):
                        lo = 512 * h_idx + 128 * m
                        nc.vector.tensor_add(
                            ps[:, lo : lo + 128], ps[:, lo : lo + 128], mask_sb[:])
                if off == 0:
                    runs = [(0, 1024)]
                else:
                    runs = [(off, 512), (512 + off, 1024)]
                for lo, hi in runs:
                    nc.scalar.activation(
                        at[:, lo:hi], ps[:, lo:hi], mybir.ActivationFunctionType.Exp)
                pending.append((j, off, at))
                if len(pending) > 2:
                    emit_pv(hp, *pending.pop(0))
                it += 1
                while emitted < nfill and emitted * iters < it * nfill:
                    fillers[emitted]()
                    emitted += 1
            for p in pending:
                emit_pv(hp, *p)
        while emitted < nfill:
            fillers[emitted]()
            emitted += 1
        # ---- batched rollout for both head pairs ----
        recip4 = small_p.tile([128, 512], dt.float32, tag="recip4", name="recip4")
        nc.vector.memset(recip4[:], 1.0)
        for i in range(4):
            nc.vector.tensor_copy(recip4[32 * i : 32 * i + 1, :], po[i][64:65, :])
        nc.vector.reciprocal(recip4[:], recip4[:])
        for hp in range(2):
            bch = small_p.tile([128, 512], dt.float32, tag=f"bc{hp}", name=f"bc{hp}")
            for h_idx in range(2):
                i = 2 * hp + h_idx
                nc.sync.dma_start(
                    bch[64 * h_idx : 64 * h_idx + 64, :],
                    recip4[32 * i : 32 * i + 1, :]
                    .rearrange("a (o n) -> a o n", o=1)
                    .to_broadcast((1, 64, 512)),
                )
            tmp = small_p.tile([128, 512], dt.float32, tag=f"tmp{hp}", name=f"tmp{hp}")
            nc.vector.tensor_mul(tmp[0:64, :], po[2 * hp][0:64, :], bch[0:64, :])
            nc.vector.tensor_mul(tmp[64:128, :], po[2 * hp + 1][0:64, :], bch[64:128, :])
            nc.vector.tensor_scalar_add(
                pairt[hp][:, 512 * c : 512 * (c + 1)], tmp[:], bv_sb[:, hp : hp + 1]
            )
        for u in tail:
            u()

# ----------------------------------------------------------------------
_PROGRAM = None


def _get_program():
    global _PROGRAM
    if _PROGRAM is None:
        _PROGRAM = _build_program()
    return _PROGRAM


def _make_in_maps(inputs, W_in, b_in, W_out, b_out):
    in_maps = []
    scale = 1.0 / np.sqrt(np.float32(HD))
    kr = np.arange(128)[:, None]
    qc = np.arange(128)[None, :]
    trimask = np.where(qc >= kr, 0.0, -1e30).astype(np.float32)
    for core in range(NC):
        b, g = divmod(core, 4)
        r = slice(256 * g, 256 * (g + 1))
        wq = W_in[0:E][r] * scale
        wk = W_in[E : 2 * E][r]
        wv = W_in[2 * E : 3 * E][r]
        xT = np.ascontiguousarray(inputs[b].T.astype(np.float32))
        wqkT = np.ascontiguousarray(np.concatenate([wq, wk], axis=0).T)
        wvT = np.ascontiguousarray(wv.T)
        bqk = np.concatenate([b_in[0:E][r] * scale, b_in[E : 2 * E][r]])
        bv = np.ascontiguousarray(b_in[2 * E : 3 * E][r])
        wo = np.ascontiguousarray(W_out[:, r].T)
        in_maps.append(
            {
                "xT": xT,
                "wqkT": wqkT.astype(np.float32),
                "wvT": wvT.astype(np.float32),
                "bqk": bqk.astype(np.float32),
                "bv": bv.astype(np.float32),
                "wo": wo.astype(np.float32),
                "trimask": trimask,
            }
        )
    return in_maps


def run_spmd(inputs, W_in, b_in, W_out, b_out, trace=False, **kw):
    nc = _get_program()
    in_maps = _make_in_maps(inputs, W_in, b_in, W_out, b_out)
    bkr = run_bass_kernel_spmd(nc, in_maps, list(range(NC)), trace=trace, **kw)
    parts = [bkr.results[i]["y"] for i in range(NC)]
    out = np.stack(
        [
            parts[0] + parts[1] + parts[2] + parts[3],
            parts[4] + parts[5] + parts[6] + parts[7],
        ]
    )
    out = out + b_out[None, None, :]
    return out.astype(np.float32), bkr


def kernel(inputs, W_in, b_in, W_out, b_out):
    out, _ = run_spmd(
        np.asarray(inputs, dtype=np.float32),
        np.asarray(W_in, dtype=np.float32),
        np.asarray(b_in, dtype=np.float32),
        np.asarray(W_out, dtype=np.float32),
        np.asarray(b_out, dtype=np.float32),
    )
    return out


# revision 42
# speedup vs baseline: 1.0106x; 1.0106x over previous
"""Trainium2 Bass kernel for causal multi-head attention (B=2, S=2048, E=1024, H=16).

Sharding: 8 cores = 2 batches x 4 head-groups (4 heads each).
Each core computes its batch's QKV for its 4 heads, causal attention, and a
partial output projection; host sums the 4 group partials per batch + b_out.

All big matmuls run in float32r (TF32-like, 1 cycle/row at N>=256).
"""
import sys

sys.path.insert(0, "/opt/trn_rl_repo")

from contextlib import ExitStack

import numpy as np

import concourse.bass as bass
import concourse.tile as tile
from concourse import bacc, mybir
from concourse.bass_utils import run_bass_kernel_spmd

dt = mybir.dt

B, S, E, H = 2, 2048, 1024, 16
HD = 64                     # head dim
HPC = 4                     # heads per core
NC = 8                      # cores
KE = E // 128               # 8 contraction k-tiles for projections
NT = S // 128               # 16 token tiles
NCH = S // 512              # 4 token chunks
FQK = 512                   # q+k features per core (4 heads * 64 * 2)
FV = 256                    # v features per core

# engine used for fp32 -> fp32r rounding copies of DMA'd inputs
ROUND_ENGINE = "gpsimd"


def _build_program():
    nc = bacc.Bacc("TRN2", target_bir_lowering=False, debug=False, num_devices=NC)

    xT_d = nc.dram_tensor("xT", [E, S], dt.float32, kind="ExternalInput")
    wqkT_d = nc.dram_tensor("wqkT", [E, FQK], dt.float32, kind="ExternalInput")
    wvT_d = nc.dram_tensor("wvT", [E, FV], dt.float32, kind="ExternalInput")
    bqk_d = nc.dram_tensor("bqk", [FQK], dt.float32, kind="ExternalInput")
    bv_d = nc.dram_tensor("bv", [FV], dt.float32, kind="ExternalInput")
    wo_d = nc.dram_tensor("wo", [FV, E], dt.float32, kind="ExternalInput")
    mask_d = nc.dram_tensor("trimask", [128, 128], dt.float32, kind="ExternalInput")
    y_d = nc.dram_tensor("y", [S, E], dt.float32, kind="ExternalOutput")

    with TileKernel(nc) as tk:
        tk.build(xT_d, wqkT_d, wvT_d, bqk_d, bv_d, wo_d, mask_d, y_d)
    nc.compile()
    return nc


class TileKernel:
    def __init__(self, nc):
        self.nc = nc
        self.ctx = ExitStack()
        self.tc_cm = tile.TileContext(nc)

    def __enter__(self):
        self.tc = self.tc_cm.__enter__()
        return self

    def __exit__(self, *a):
        self.ctx.close()
        return self.tc_cm.__exit__(*a)

    _round_i = 0

    def round_copy(self, out, in_):
        nc = self.nc
        nc.vector.tensor_copy(out, in_)

    def build(self, xT_d, wqkT_d, wvT_d, bqk_d, bv_d, wo_d, mask_d, y_d):
        nc, tc, ctx = self.nc, self.tc, self.ctx
        pool = lambda name, bufs, **kw: ctx.enter_context(
            tc.tile_pool(name=name, bufs=bufs, **kw)
        )

        const_p = pool("const", 1)
        xs_p = pool("xs", 1)
        xr_p = pool("xr", 2)
        qkt_p = pool("qkt", 1)
        vones_p = pool("vones", 1)
        attn_p = pool("attn", 3)
        pair_p = pool("pair", 1)
        small_p = pool("small", 1)
        y_p = pool("y", 4)
        # PSUM: ps (2 banks x 2 bufs) + po (1 bank x 1 buf x 4 tags) = 8
        ps_p = pool("ps", 2, space="PSUM")
        po_p = pool("po", 1, space="PSUM")
        p1_p = ps_p  # qkv/outproj psums share the ps slots

        # ---- weights ----

        wstage_cm = tc.tile_pool(name="wstage", bufs=1)
        wstage_p = wstage_cm.__enter__()
        wqk_st = wstage_p.tile([128, KE * FQK], dt.float32, tag="wst")
        wqk_big = const_p.tile([128, KE * FQK], dt.float32r, tag="wqk")
        for h in range(2):
            hs = slice(h * (KE // 2) * FQK, (h + 1) * (KE // 2) * FQK)
            nc.sync.dma_start(
                wqk_st[:, hs].rearrange("p (ke f) -> p ke f", f=FQK),
                wqkT_d[h * 512 : (h + 1) * 512, :].rearrange("(ke p) f -> p ke f", p=128),
            )
            self.round_copy(wqk_big[:, hs], wqk_st[:, hs])
        wqk_r = [wqk_big[:, FQK * ke : FQK * (ke + 1)] for ke in range(KE)]

        wv_st = wstage_p.tile([128, KE * FV], dt.float32, tag="wst")
        nc.sync.dma_start(
            wv_st[:].rearrange("p (ke f) -> p ke f", f=FV),
            wvT_d[:].rearrange("(ke p) f -> p ke f", p=128),
        )
        wv_big = const_p.tile([128, KE * FV], dt.float32r, tag="wv")
        self.round_copy(wv_big[:], wv_st[:])
        wv_r = [wv_big[:, FV * ke : FV * (ke + 1)] for ke in range(KE)]

        wo_st = wstage_p.tile([128, 2 * E], dt.float32, tag="wst")
        nc.sync.dma_start(
            wo_st[:].rearrange("p (kt f) -> p kt f", f=E),
            wo_d[:].rearrange("(kt p) f -> p kt f", p=128),
        )
        wo_big = const_p.tile([128, 2 * E], dt.float32r, tag="wo")
        self.round_copy(wo_big[:], wo_st[:])
        wo_r = [wo_big[:, E * kt : E * (kt + 1)] for kt in range(2)]
        wstage_cm.__exit__(None, None, None)

        bqk_sb = const_p.tile([128, 4], dt.float32, tag="bqk")
        nc.sync.dma_start(bqk_sb[:], bqk_d[:].rearrange("(f p) -> p f", p=128))
        bv_sb = const_p.tile([128, 2], dt.float32, tag="bv")
        nc.sync.dma_start(bv_sb[:], bv_d[:].rearrange("(f p) -> p f", p=128))
        ones_sb = const_p.tile([128, 1, 1], dt.float32, tag="ones")
        nc.vector.memset(ones_sb[:], 1.0)
        mask_sb = const_p.tile([128, 128], dt.float32, tag="mask")
        nc.sync.dma_start(mask_sb[:], mask_d[:])

        # ---- persistent activations ----
        # qkt tiles: 0: q heads 0,1 | 1: q heads 2,3 | 2: k heads 0,1 | 3: k heads 2,3
        qkt = [qkt_p.tile([128, S], dt.float32r, tag=f"qkt{f}", name=f"qkt{f}") for f in range(4)]
        # vones[t]: [v h0 |1| v h1 |1| v h2 |1| v h3 |1] for token tile t
        vones = [vones_p.tile([128, 4 * 65], dt.float32r, tag=f"v{t}", name=f"v{t}") for t in range(NT)]
        # pair tiles: final normalized attn output, [head dims x 2, S]
        pairt = [pair_p.tile([128, S], dt.float32r, tag=f"pair{hp}", name=f"pair{hp}") for hp in range(2)]

        env = dict(
            xT_d=xT_d, wqk_r=wqk_r, wv_r=wv_r, bqk_sb=bqk_sb, ones_sb=ones_sb,
            xs_p=xs_p, xr_p=xr_p, p1_p=p1_p, qkt=qkt, vones=vones,
            ps_p=ps_p, po_p=po_p, attn_p=attn_p, small_p=small_p,
            pairt=pairt, bv_sb=bv_sb, mask_sb=mask_sb, wo_r=wo_r,
            y_p=y_p, y_d=y_d,
        )
        # startup: chunk-0 qkv emitted directly
        for u in self.qkv_units(0, env):
            u()
        for c in range(NCH):
            fillers = []
            if c + 1 < NCH:
                fillers += self.qkv_units(c + 1, env)
            tail = self.oproj_units(c - 1, env) if c >= 1 else []
            self.attention_chunk(c, env, fillers, tail)
        for u in self.oproj_units(NCH - 1, env):
            u()

    # ------------------------------------------------------------------
    def qkv_units(self, c, env):
        nc = self.nc
        cs = slice(512 * c, 512 * (c + 1))
        xT_d, wqk_r, wv_r = env["xT_d"], env["wqk_r"], env["wv_r"]
        qkt, vones = env["qkt"], env["vones"]
        bqk_sb, ones_sb = env["bqk_sb"], env["ones_sb"]
        xs_p, xr_p, p1_p = env["xs_p"], env["xr_p"], env["p1_p"]

        xs = xs_p.tile([128, KE * 512], dt.float32, tag="xs", name="xs")
        nc.sync.dma_start(
            xs[:].rearrange("p (ke f) -> p ke f", f=512),
            xT_d[:, cs].rearrange("(ke p) f -> p ke f", p=128),
        )
        xrb = xr_p.tile([128, KE * 512], dt.float32r, tag="xr", name="xrb")
        xr = [xrb[:, 512 * ke : 512 * (ke + 1)] for ke in range(KE)]
        units = []
        for ke in range(KE):
            units.append(lambda ke=ke: self.round_copy(
                xrb[:, 512 * ke : 512 * (ke + 1)], xs[:, 512 * ke : 512 * (ke + 1)]))

        def qk_unit(f):
            pq = p1_p.tile([128, 1024], dt.float32, tag="ps", name="pq")
            for ke in range(KE):
                nc.tensor.matmul(
                    pq[:, 0:512], wqk_r[ke][:, 128 * f : 128 * (f + 1)], xr[ke][:],
                    start=(ke == 0), stop=(ke == KE - 1),
                )
            nc.vector.tensor_scalar_add(qkt[f][:, cs], pq[:, 0:512], bqk_sb[:, f : f + 1])

        def v_unit(t4):
            t = 4 * c + t4
            pv = p1_p.tile([128, 1024], dt.float32, tag="ps", name="pv")
            for ke in range(KE):
                nc.tensor.matmul(
                    pv[:, 0:FV],
                    xr[ke][:, 128 * t4 : 128 * (t4 + 1)], wv_r[ke][:],
                    start=(ke == 0), stop=(ke == KE - 1),
                )
            vt = vones[t]
            v3 = vt[:].rearrange("p (g d) -> p g d", d=65)
            nc.vector.tensor_copy(
                v3[:, :, 0:64],
                pv[:, 0:FV].rearrange("p (g d) -> p g d", d=64),
            )
            nc.vector.tensor_copy(v3[:, :, 64:65], ones_sb[:].to_broadcast((128, 4, 1)))

        for f in range(4):
            units.append(lambda f=f: qk_unit(f))
        for t4 in range(4):
            units.append(lambda t4=t4: v_unit(t4))
        return units

    # ------------------------------------------------------------------
    def oproj_units(self, c, env):
        nc = self.nc
        pairt, wo_r, p1_p, y_p, y_d = (
            env["pairt"], env["wo_r"], env["p1_p"], env["y_p"], env["y_d"])
        units = []
        ysbs = {}

        def unit(t4, o):
            t = 4 * c + t4
            if o == 0:
                ysbs[t4] = y_p.tile([128, E], dt.float32, tag="y", name="ysb")
            ysb = ysbs[t4]
            py = p1_p.tile([128, 1024], dt.float32, tag="ps", name="py")
            for kt in range(2):
                nc.tensor.matmul(
                    py[:, 0:512],
                    pairt[kt][:, 128 * t : 128 * (t + 1)],
                    wo_r[kt][:, 512 * o : 512 * (o + 1)],
                    start=(kt == 0), stop=(kt == 1),
                )
            if o == 0:
                nc.vector.tensor_copy(ysb[:, 0:512], py[:, 0:512])
            else:
                nc.scalar.activation(
                    ysb[:, 512:1024], py[:, 0:512], mybir.ActivationFunctionType.Copy
                )
                eng = nc.gpsimd if t % 2 == 0 else nc.sync
                eng.dma_start(y_d[128 * t : 128 * (t + 1), :], ysb[:])

        for t4 in range(4):
            for o in range(2):
                units.append(lambda t4=t4, o=o: unit(t4, o))
        return units

    # ------------------------------------------------------------------
    def attention_chunk(self, c, env, fillers, tail=()):
        """Attention for both head pairs of chunk c, weaving filler units
        (next-chunk qkv / prev-chunk out-proj) into the PE stream."""
        nc = self.nc
        qkt, vones = env["qkt"], env["vones"]
        ps_p, po_p, attn_p, small_p = (
            env["ps_p"], env["po_p"], env["attn_p"], env["small_p"])
        pairt, bv_sb, mask_sb = env["pairt"], env["bv_sb"], env["mask_sb"]
        nj = 4 * c + 4
        # po[2*hp + h_idx]: [65, 512] accumulator per head
        po = [po_p.tile([65, 512], dt.float32, tag=f"po{i}", name=f"po{i}")
              for i in range(4)]

        nfill = len(fillers)
        iters = 2 * nj
        emitted = 0

        def emit_pv(hp, j, off, at):
            for h_idx in range(2):
                slot = 2 * hp + h_idx
                nc.tensor.matmul(
                    po[slot][:, off:512],
                    vones[j][:, 65 * slot : 65 * slot + 65],
                    at[:, 512 * h_idx + off : 512 * (h_idx + 1)],
                    start=(j == 0), stop=(j == nj - 1),
                    skip_group_check=True,
                )

        it = 0
        for hp in range(2):
            pending = []
            for j in range(nj):
                ps = ps_p.tile([128, 1024], dt.float32, tag="ps", name="ps")
                at = attn_p.tile([128, 1024], dt.float32r, tag="attn", name="at")
                m = j - 4 * c
                off = 128 * m if 1 <= m <= 3 else 0
                off_mm = off if m in (1, 2) else 0
                for h_idx in range(2):
                    r0 = 64 * h_idx
                    nc.tensor.matmul(
                        ps[:, 512 * h_idx + off_mm : 512 * (h_idx + 1)],
                        qkt[2 + hp][r0 : r0 + 64, 128 * j : 128 * (j + 1)],
                        qkt[hp][r0 : r0 + 64, 512 * c + off_mm : 512 * (c + 1)],
                        start=True, stop=True,
                    )
                if off == 0:
                    runs = [(0, 1024)]
                else:
                    runs = [(off, 512), (512 + off, 1024)]
                for lo, hi in runs:
                    nc.scalar.activation(
                        at[:, lo:hi], ps[:, lo:hi], mybir.ActivationFunctionType.Exp)
                if m >= 0:
                    for h_idx in range(2):
                        lo = 512 * h_idx + 128 * m
                        nc.vector.tensor_mul(
                            at[:, lo : lo + 128], at[:, lo : lo + 128], mask_sb[:])
                pending.append((j, off, at))
                if len(pending) > 2:
                    emit_pv(hp, *pending.pop(0))
                it += 1
                while emitted < nfill and emitted * iters < it * nfill:
                    fillers[emitted]()
                    emitted += 1
            for p in pending:
                emit_pv(hp, *p)
        while emitted < nfill:
            fillers[emitted]()
            emitted += 1
        # ---- batched rollout for both head pairs ----
        recip4 = small_p.tile([128, 512], dt.float32, tag="recip4", name="recip4")
        nc.vector.memset(recip4[:], 1.0)
        for i in range(4):
            nc.vector.tensor_copy(recip4[32 * i : 32 * i + 1, :], po[i][64:65, :])
        nc.vector.reciprocal(recip4[:], recip4[:])
        for hp in range(2):
            bch = small_p.tile([128, 512], dt.float32, tag=f"bc{hp}", name=f"bc{hp}")
            for h_idx in range(2):
                i = 2 * hp + h_idx
                nc.sync.dma_start(
                    bch[64 * h_idx : 64 * h_idx + 64, :],
                    recip4[32 * i : 32 * i + 1, :]
                    .rearrange("a (o n) -> a o n", o=1)
                    .to_broadcast((1, 64, 512)),
                )
            tmp = small_p.tile([128, 512], dt.float32, tag=f"tmp{hp}", name=f"tmp{hp}")
            nc.vector.tensor_mul(tmp[0:64, :], po[2 * hp][0:64, :], bch[0:64, :])
            nc.vector.tensor_mul(tmp[64:128, :], po[2 * hp + 1][0:64, :], bch[64:128, :])
            nc.vector.tensor_scalar_add(
                pairt[hp][:, 512 * c : 512 * (c + 1)], tmp[:], bv_sb[:, hp : hp + 1]
            )
        for u in tail:
            u()

# ----------------------------------------------------------------------
_PROGRAM = None


def _get_program():
    global _PROGRAM
    if _PROGRAM is None:
        _PROGRAM = _build_program()
    return _PROGRAM


def _make_in_maps(inputs, W_in, b_in, W_out, b_out):
    in_maps = []
    scale = 1.0 / np.sqrt(np.float32(HD))
    kr = np.arange(128)[:, None]
    qc = np.arange(128)[None, :]
    trimask = np.where(qc >= kr, 1.0, 0.0).astype(np.float32)
    for core in range(NC):
        b, g = divmod(core, 4)
        r = slice(256 * g, 256 * (g + 1))
        wq = W_in[0:E][r] * scale
        wk = W_in[E : 2 * E][r]
        wv = W_in[2 * E : 3 * E][r]
        xT = np.ascontiguousarray(inputs[b].T.astype(np.float32))
        wqkT = np.ascontiguousarray(np.concatenate([wq, wk], axis=0).T)
        wvT = np.ascontiguousarray(wv.T)
        bqk = np.concatenate([b_in[0:E][r] * scale, b_in[E : 2 * E][r]])
        bv = np.ascontiguousarray(b_in[2 * E : 3 * E][r])
        wo = np.ascontiguousarray(W_out[:, r].T)
        in_maps.append(
            {
                "xT": xT,
                "wqkT": wqkT.astype(np.float32),
                "wvT": wvT.astype(np.float32),
                "bqk": bqk.astype(np.float32),
                "bv": bv.astype(np.float32),
                "wo": wo.astype(np.float32),
                "trimask": trimask,
            }
        )
    return in_maps


def run_spmd(inputs, W_in, b_in, W_out, b_out, trace=False, **kw):
    nc = _get_program()
    in_maps = _make_in_maps(inputs, W_in, b_in, W_out, b_out)
    bkr = run_bass_kernel_spmd(nc, in_maps, list(range(NC)), trace=trace, **kw)
    parts = [bkr.results[i]["y"] for i in range(NC)]
    out = np.stack(
        [
            parts[0] + parts[1] + parts[2] + parts[3],
            parts[4] + parts[5] + parts[6] + parts[7],
        ]
    )
    out = out + b_out[None, None, :]
    return out.astype(np.float32), bkr


def kernel(inputs, W_in, b_in, W_out, b_out):
    out, _ = run_spmd(
        np.asarray(inputs, dtype=np.float32),
        np.asarray(W_in, dtype=np.float32),
        np.asarray(b_in, dtype=np.float32),
        np.asarray(W_out, dtype=np.float32),
        np.asarray(b_out, dtype=np.float32),
    )
    return out
